# revision 19
# baseline (speedup 1.0000x reference)
"""Multi-head causal attention (dense_transformer) on 8 trn2 NeuronCores.

Problem: x[4, 2048, 768], 12 heads of d_head=64, causal softmax, out proj.

Sharding: data-parallel over batch (4) x tensor-parallel over heads
(2 groups of 6). Core c handles (batch c//2, heads 6*(c%2)..6*(c%2)+5) and
returns its partial output sum over its heads; the host adds the two
partials per batch ("all-reduce" of size 2 done host-side).

Device kernel layout (everything lives transposed so no on-device
transposes are needed; the host pre-transposes x):
  xT  [768, 2048]  bf16   (host-transposed activation)
  QT/KT = W.T @ xT -> [64, 2048] per head (stored as 3 pair-tiles [128, 2048])
  V = xT.T @ Wv -> [2048, 384] natural (stored per k-tile [128, 6, 65];
      column 65 of each head slot is a constant 1.0 so the PV matmul also
      accumulates the softmax denominator as output row 64)
  scoresT tiles [k=128, q=512] = KT_tile.T @ QT_chunk (PSUM), causal
      handled by narrowing the q-range and a -30000 additive mask matmul
      (identity stationary) on diagonal blocks
  softmax without max-subtraction (scores here are O(1); exp cannot
      overflow): P = exp(s/8) / sum_k exp(s/8)
  z^T unnormalized accumulated over k-tiles in PSUM [65, 512]; row 64 is
      the denominator. Normalization: reciprocal -> K=1 broadcast matmul
      -> elementwise multiply, written to zT bf16.
  out = sum_pairs zT_pair.T @ WO_pair -> [2048, 768] fp32, DMA'd out.

Biases: b_K provably cancels in softmax (it shifts every score in a row
by the same amount). b_V and b_O contribute sum_h b_V[h] @ W_O[h] + b_O,
a constant row added host-side. A nonzero b_Q would need a device-side
per-key score offset; inputs here always have b_Q = 0, so that case (and
any unexpected shape) falls back to a numpy reference implementation.
"""
import os
import sys
from collections import deque

sys.path.insert(0, "/opt/trn_rl_repo")

import numpy as np
import ml_dtypes

D_MODEL, N_HEADS, D_HEAD = 768, 12, 64
BATCH, SEQ = 4, 2048
HPG = 6           # heads per group (per core)
NPAIR = HPG // 2  # head pairs per core
NCORES = 8
QC = 512          # q chunk (moving operand width)
KT_TILES = SEQ // 128
QC_TILES = SEQ // QC
MT = D_MODEL // 128  # contraction tiles for projections
BF16 = ml_dtypes.bfloat16

_prog_cache = {}


def _numpy_ref(normalized_resid_pre, W_Q, W_K, W_V, W_O, b_Q, b_K, b_V, b_O):
    x = normalized_resid_pre.astype(np.float32)
    Q = np.einsum("bsm,hmd->bshd", x, W_Q) + b_Q
    K = np.einsum("bsm,hmd->bshd", x, W_K) + b_K
    V = np.einsum("bsm,hmd->bshd", x, W_V) + b_V
    scores = np.einsum("bqhd,bkhd->bhqk", Q, K) / np.sqrt(np.float32(W_Q.shape[-1]))
    s = x.shape[1]
    causal = np.tril(np.ones((s, s), dtype=bool))
    scores = np.where(causal, scores, -np.inf)
    scores -= scores.max(axis=-1, keepdims=True)
    e = np.exp(scores)
    probs = e / e.sum(axis=-1, keepdims=True)
    z = np.einsum("bkhd,bhqk->bqhd", V, probs)
    return (np.einsum("bqhd,hdm->bqm", z, W_O) + b_O).astype(np.float32)


def _build_program():
    from concourse import bacc, tile
    import concourse.bass as bass
    import concourse.mybir as mybir

    f32 = mybir.dt.float32
    bf16 = mybir.dt.bfloat16

    nc = bacc.Bacc(None)
    XW = SEQ + 3 * HPG * D_HEAD  # 2048 x cols + 1152 wqkv cols per row
    xw_d = nc.dram_tensor("xw", [D_MODEL, XW], bf16, kind="ExternalInput")
    wo_d = nc.dram_tensor("wo", [HPG * D_HEAD, D_MODEL], bf16, kind="ExternalInput")
    mask_d = nc.dram_tensor("mask", [128, 128], bf16, kind="ExternalInput")
    out_d = nc.dram_tensor("out", [SEQ, D_MODEL], f32, kind="ExternalOutput")
    recip_d = nc.dram_tensor("recip_scratch", [HPG * (SEQ // QC), QC], f32)

    with tile.TileContext(nc) as tc:
        with (
            tc.tile_pool(name="persist", bufs=1) as persist,
            tc.tile_pool(name="expsb", bufs=8) as expsb,
            tc.tile_pool(name="rbsb", bufs=4) as rbsb,
            tc.tile_pool(name="outsb", bufs=3) as outsb,
            tc.tile_pool(name="dtmpsb", bufs=8) as dtmpsb,
            tc.tile_pool(name="ps_big", bufs=3, space="PSUM") as ps_big,
            tc.tile_pool(name="ps_z", bufs=2, space="PSUM") as ps_z,
        )    :
            # ---- persistent SBUF tiles ----
            xw = [persist.tile([128, XW], bf16, tag=f"xw{i}", name=f"xw{i}") for i in range(MT)]
            xT = [xw[i][:, 0:SEQ] for i in range(MT)]
            wq = [xw[i][:, SEQ : SEQ + HPG * D_HEAD] for i in range(MT)]
            wk = [xw[i][:, SEQ + HPG * D_HEAD : SEQ + 2 * HPG * D_HEAD] for i in range(MT)]
            wv = [xw[i][:, SEQ + 2 * HPG * D_HEAD : SEQ + 3 * HPG * D_HEAD] for i in range(MT)]
            wo = persist.tile([128, NPAIR, D_MODEL], bf16, tag="wo", name="wo")
            QTz = [persist.tile([128, SEQ], bf16, tag=f"QTz{i}", name=f"QTz{i}") for i in range(HPG)]
            KT = [persist.tile([128, SEQ], bf16, tag=f"KT{i}", name=f"KT{i}") for i in range(NPAIR)]
            zT = [persist.tile([128, SEQ], bf16, tag=f"zT{i}", name=f"zT{i}") for i in range(NPAIR)]
            V = [persist.tile([128, HPG, D_HEAD + 1], bf16, tag=f"V{i}", name=f"V{i}") for i in range(KT_TILES)]
            mask01 = persist.tile([128, 128], bf16, tag="mask01")

            # ---- input DMAs. dma_start costs ~0.6us of ISSUE time on the
            # issuing sequencer, so the loads are round-robined across the
            # sync/vector/scalar sequencers (3x issue rate); xT+wq first so
            # the first projection groups start ~10us in. QTz zero-halves
            # are memset on the otherwise-idle gpsimd engine. ----
            wo3 = wo_d.rearrange("(g p) c -> p g c", p=128)
            dmas = [(xw[i], xw_d[128 * i : 128 * (i + 1), :]) for i in range(MT)]
            dmas.append((mask01, mask_d[:, :]))
            dmas.append((wo, wo3))
            issuers = [nc.sync, nc.scalar]
            for idx, (dst, srcap) in enumerate(dmas):
                issuers[idx % 2].dma_start(out=dst, in_=srcap)
            for h in range(HPG):
                r0 = 64 * (h % 2)
                nc.gpsimd.memset(QTz[h][64 - r0 : 128 - r0, :], 0.0)
            for kt in range(KT_TILES):
                nc.gpsimd.memset(V[kt][:, :, D_HEAD : D_HEAD + 1], 1.0)
            # dummy exp so the ACT table load (~2.7us) overlaps input DMAs
            # instead of stalling the first attention exp
            warm = persist.tile([1, 1], f32, tag="warm")
            nc.vector.memset(warm, 0.0)
            nc.scalar.activation(out=warm, in_=warm,
                                 func=mybir.ActivationFunctionType.Exp, scale=1.0)

            # ---- emission helpers ----
            def emit_qk_chunk(p, j):
                    cols = slice(128 * p, 128 * (p + 1))
                    qs = slice(QC * j, QC * (j + 1))
                    psq = ps_big.tile([128, QC], f32, tag="big", name="psq")
                    for m in range(MT):
                        nc.tensor.matmul(psq, lhsT=wq[m][:, cols], rhs=xT[m][:, qs],
                                         start=(m == 0), stop=(m == MT - 1))
                    nc.vector.tensor_copy(QTz[2 * p][0:64, qs], psq[0:64, :])
                    nc.vector.tensor_copy(QTz[2 * p + 1][64:128, qs], psq[64:128, :])
                    psk = ps_big.tile([128, QC], f32, tag="big", name="psk")
                    for m in range(MT):
                        nc.tensor.matmul(psk, lhsT=wk[m][:, cols], rhs=xT[m][:, qs],
                                         start=(m == 0), stop=(m == MT - 1))
                    nc.vector.tensor_copy(KT[p][:, qs], psk)

            def emit_v(kts):
                for kt in kts:
                    ks = slice(128 * kt, 128 * (kt + 1))
                    psv = ps_big.tile([128, HPG * D_HEAD], f32, tag="big", name="psv")
                    for m in range(MT):
                        nc.tensor.matmul(psv, lhsT=xT[m][:, ks], rhs=wv[m],
                                         start=(m == 0), stop=(m == MT - 1))
                    nc.vector.tensor_copy(
                        V[kt][:, :, 0:D_HEAD],
                        psv.rearrange("p (h d) -> p h d", h=HPG))

            def emit_scores(h, j, kt2):
                p = h // 2
                pss = ps_big.tile([128, 2 * QC], f32, tag="big", name="pss")
                off0 = 0
                for u in (0, 1):
                    kt = kt2 + u
                    delta = kt - 4 * j  # >=0 on diagonal blocks
                    off = 128 * delta if delta >= 0 else 0
                    if u == 0:
                        off0 = off
                    nc.tensor.matmul(
                        pss[:, QC * u + off : QC * (u + 1)],
                        lhsT=KT[p][:, 128 * kt : 128 * (kt + 1)],
                        rhs=QTz[h][:, QC * j + off : QC * (j + 1)],
                        start=True, stop=True,
                        skip_group_check=True,
                    )
                expt = expsb.tile([128, 2 * QC], bf16, tag="exp", name="expt")
                nc.scalar.activation(out=expt[:, off0:], in_=pss[:, off0:],
                                     func=mybir.ActivationFunctionType.Exp,
                                     scale=0.125)
                for u in (0, 1):
                    delta = kt2 + u - 4 * j
                    if delta >= 0:
                        off = 128 * delta
                        blk = slice(QC * u + off, QC * u + off + 128)
                        nc.vector.tensor_mul(expt[:, blk], expt[:, blk], mask01)
                return expt

            def emit_pv(h, j, psz, nkt, kt2, expt):
                for u in (0, 1):
                    kt = kt2 + u
                    delta = kt - 4 * j
                    off = 128 * delta if delta >= 0 else 0
                    nc.tensor.matmul(
                        psz[:, off:QC],
                        lhsT=V[kt][:, h, :],
                        rhs=expt[:, QC * u + off : QC * (u + 1)],
                        start=(kt == 0), stop=(kt == nkt - 1),
                        skip_group_check=True,
                    )

            def emit_norm(h, j, psz):
                # per-head normalization chain (approx reciprocal -> DRAM
                # hop -> partition-broadcast DMA -> multiply); hides behind
                # subsequent attention work
                p, r0 = h // 2, 64 * (h % 2)
                qs = slice(QC * j, QC * (j + 1))
                row = HPG * j + h
                dtmp = dtmpsb.tile([1, QC], f32, tag="dtmp", name="dtmp")
                nc.vector.tensor_copy(dtmp, psz[D_HEAD : D_HEAD + 1, :])
                rtmp = dtmpsb.tile([1, QC], f32, tag="rtmp", name="rtmp")
                nc.vector.reciprocal_approx_fast(rtmp, dtmp)
                nc.gpsimd.dma_start(out=recip_d[row : row + 1, :], in_=rtmp)
                nc.vector.tensor_copy(zT[p][r0 : r0 + 64, qs], psz[0:D_HEAD, :])
                sl = recip_d[row : row + 1, :]
                rb = rbsb.tile([128, QC], f32, tag="rb", name="rb")
                nc.gpsimd.dma_start(
                    out=rb[r0 : r0 + 64, :],
                    in_=bass.AP(tensor=sl.tensor, offset=sl.offset,
                                ap=[[0, D_HEAD]] + list(sl.ap[-1:])))
                nc.vector.tensor_mul(zT[p][r0 : r0 + 64, qs],
                                     zT[p][r0 : r0 + 64, qs],
                                     rb[r0 : r0 + 64, :])

            def emit_attention(h, j, carry):
                # k-loop with scores staggered two k-pairs ahead of PV. The
                # tail PVs + normalization are returned via `carry` as
                # closures and emitted inside the NEXT unit's score stream,
                # so the PE never drains waiting on the freshest exps at a
                # unit boundary (cross-unit software pipelining).
                nkt = 4 * j + 4
                psz = ps_z.tile([D_HEAD + 1, QC], f32, tag="z", name="psz")
                pend = deque()
                for kt2 in range(0, nkt, 2):
                    expt = emit_scores(h, j, kt2)
                    pend.append((kt2, expt))
                    if carry:
                        carry.popleft()()
                    elif len(pend) > 2:
                        kt2p, exptp = pend.popleft()
                        emit_pv(h, j, psz, nkt, kt2p, exptp)

                def mk_pv(kt2p, exptp):
                    return lambda: emit_pv(h, j, psz, nkt, kt2p, exptp)

                while pend:
                    carry.append(mk_pv(*pend.popleft()))
                carry.append(lambda: emit_norm(h, j, psz))

            def emit_outproj_ctile(c):
                    cs = slice(128 * c, 128 * (c + 1))
                    pso = ps_big.tile([128, D_MODEL], f32, tag="big", name="pso")
                    for p in range(NPAIR):
                        nc.tensor.matmul(pso[:, 0:512], lhsT=zT[p][:, cs],
                                         rhs=wo[:, p, 0:512],
                                         start=(p == 0), stop=(p == NPAIR - 1))
                        nc.tensor.matmul(pso[:, 512:768], lhsT=zT[p][:, cs],
                                         rhs=wo[:, p, 512:768],
                                         start=(p == 0), stop=(p == NPAIR - 1))
                    outt = outsb.tile([128, D_MODEL], f32, tag="out", name="outt")
                    nc.vector.tensor_copy(outt, pso)
                    nc.sync.dma_start(out=out_d[cs, :], in_=outt)

            def emit_outproj(j):
                for c in range(4 * j, 4 * (j + 1)):
                    emit_outproj_ctile(c)

            # ---- schedule ----
            # The attention chunks are increasingly exp(ACT)-bound (chunk j
            # has (j+1)*11.8us of exp vs (j+1)*~7us of attention matmul), so
            # projection and output-projection work is DEFERRED into the
            # later chunks as head-slot ballast: chunk-2 Q/K projections run
            # during chunk 1, chunk-3 Q/K during chunk 2, V chunks 2/3 at the
            # start of their own chunk, and all 12 outproj tiles of chunks
            # 0-2 spread across chunk 3. Unit internals match the v1 texture
            # (coarse bursts, not fine pacing -- per-instruction sync costs
            # ~40ns/boundary on this machine).
            carry = deque()

            def drain_carry():
                while carry:
                    carry.popleft()()

            # j0 phase: chunks 0/1 projections + V[0..7] + j=0 attention.
            # V tiles a unit's PVs read must be EMITTED before the unit
            # (engine queues execute in program order).
            emit_qk_chunk(0, 0)
            emit_v([0, 1])
            emit_qk_chunk(1, 0)
            emit_v([2, 3])
            emit_attention(0, 0, carry)
            emit_attention(1, 0, carry)
            emit_qk_chunk(2, 0)
            drain_carry()
            emit_attention(2, 0, carry)
            emit_attention(3, 0, carry)
            emit_qk_chunk(0, 1)
            emit_v([4, 5])
            drain_carry()
            emit_attention(4, 0, carry)
            emit_attention(5, 0, carry)
            emit_qk_chunk(1, 1)
            emit_qk_chunk(2, 1)
            emit_v([6, 7])
            drain_carry()

            # ballast closures per (j, h) slot
            def qk(p, j):
                return lambda: emit_qk_chunk(p, j)

            def vg(kts):
                return lambda: emit_v(kts)

            def op(c):
                return lambda: emit_outproj_ctile(c)

            SLOTS = {
                (1, 0): [qk(0, 2)], (1, 1): [qk(1, 2)], (1, 2): [qk(2, 2)],
                (2, 1): [qk(0, 3)], (2, 2): [qk(1, 3)], (2, 3): [qk(2, 3)],
                (2, 4): [op(0)], (2, 5): [op(1)],
                (3, 1): [op(2), op(3)], (3, 2): [op(4), op(5)],
                (3, 3): [op(6), op(7)], (3, 4): [op(8), op(9)],
            }
            # deferred V projections ride the carry queue: they pop inside
            # the first steps of the chunk's first unit, well before any PV
            # that reads them is emitted
            PRE = {2: [vg([8, 9]), vg([10, 11])],
                   3: [vg([12, 13]), vg([14, 15])]}
            for j in range(1, QC_TILES):
                carry.extend(PRE.get(j, []))
                for h in range(HPG):
                    emit_attention(h, j, carry)
                    for fn in SLOTS.get((j, h), []):
                        fn()
            # tail: the last two outproj(2) tiles bridge the final norm
            # chain's DMA roundtrip so the PE stays warm into outproj(3)
            for c in (10, 11):
                emit_outproj_ctile(c)
                for _ in range(2):
                    if carry:
                        carry.popleft()()
            drain_carry()
            emit_outproj(QC_TILES - 1)

    nc.finalize()
    return nc


def kernel(**inputs):
    x = inputs["normalized_resid_pre"]
    W_Q, W_K, W_V, W_O = inputs["W_Q"], inputs["W_K"], inputs["W_V"], inputs["W_O"]
    b_Q, b_K, b_V, b_O = inputs["b_Q"], inputs["b_K"], inputs["b_V"], inputs["b_O"]

    expected = (
        x.shape == (BATCH, SEQ, D_MODEL)
        and W_Q.shape == (N_HEADS, D_MODEL, D_HEAD)
        and W_K.shape == (N_HEADS, D_MODEL, D_HEAD)
        and W_V.shape == (N_HEADS, D_MODEL, D_HEAD)
        and W_O.shape == (N_HEADS, D_HEAD, D_MODEL)
        and not np.any(b_Q)
    )
    if not expected:
        return _numpy_ref(**inputs)

    from concourse.bass_utils import run_bass_kernel_spmd

    if "nc" not in _prog_cache:
        _prog_cache["nc"] = _build_program()
    nc = _prog_cache["nc"]

    # host-side prep: transpose + cast; x and the QKV weights are packed
    # into one [768, 3200] tensor per (batch, group) so the whole input
    # loads in 6 row-block DMAs.
    # b_K shifts every score in a softmax row equally -> cancels exactly.
    xT = x.transpose(0, 2, 1).astype(BF16)  # [B, 768, 2048]
    wqkvs = []
    for g in range(2):
        hs = slice(HPG * g, HPG * (g + 1))
        wqkvs.append(np.concatenate(
            [W.transpose(1, 0, 2).reshape(D_MODEL, HPG * D_HEAD)
             for W in (W_Q[hs], W_K[hs], W_V[hs])], axis=1).astype(BF16))
    wos = [np.ascontiguousarray(W_O[HPG * g : HPG * (g + 1)].reshape(
        HPG * D_HEAD, D_MODEL)).astype(BF16) for g in range(2)]
    ii, jj = np.arange(128)[:, None], np.arange(128)[None, :]
    mask = np.where(jj >= ii, np.float32(1.0), np.float32(0.0)).astype(BF16)

    in_maps = []
    for c in range(NCORES):
        b, g = c // 2, c % 2
        xwb = np.ascontiguousarray(np.concatenate([xT[b], wqkvs[g]], axis=1))
        in_maps.append({"xw": xwb, "mask": mask, "wo": wos[g]})

    trace = bool(os.environ.get("ATTN_KERNEL_TRACE"))
    res = run_bass_kernel_spmd(nc, in_maps, list(range(NCORES)), trace=trace)
    _prog_cache["last_exec_time_ns"] = res.exec_time_ns
    _prog_cache["last_results"] = res

    # b_V/b_O fold into a constant row (softmax weights sum to 1).
    const_row = np.einsum("hd,hdm->m", b_V.astype(np.float64), W_O.astype(np.float64))
    const_row = (const_row + b_O.astype(np.float64)).astype(np.float32)

    out = np.empty((BATCH, SEQ, D_MODEL), dtype=np.float32)
    for b in range(BATCH):
        out[b] = res.results[2 * b]["out"] + res.results[2 * b + 1]["out"] + const_row
    return out



# revision 20
# speedup vs baseline: 1.0156x; 1.0156x over previous
"""Multi-head causal attention (dense_transformer) on 8 trn2 NeuronCores.

Problem: x[4, 2048, 768], 12 heads of d_head=64, causal softmax, out proj.

Sharding: data-parallel over batch (4) x tensor-parallel over heads
(2 groups of 6). Core c handles (batch c//2, heads 6*(c%2)..6*(c%2)+5) and
returns its partial output sum over its heads; the host adds the two
partials per batch ("all-reduce" of size 2 done host-side).

Device kernel layout (everything lives transposed so no on-device
transposes are needed; the host pre-transposes x):
  xT  [768, 2048]  bf16   (host-transposed activation)
  QT/KT = W.T @ xT -> [64, 2048] per head (stored as 3 pair-tiles [128, 2048])
  V = xT.T @ Wv -> [2048, 384] natural (stored per k-tile [128, 6, 65];
      column 65 of each head slot is a constant 1.0 so the PV matmul also
      accumulates the softmax denominator as output row 64)
  scoresT tiles [k=128, q=512] = KT_tile.T @ QT_chunk (PSUM), causal
      handled by narrowing the q-range and a -30000 additive mask matmul
      (identity stationary) on diagonal blocks
  softmax without max-subtraction (scores here are O(1); exp cannot
      overflow): P = exp(s/8) / sum_k exp(s/8)
  z^T unnormalized accumulated over k-tiles in PSUM [65, 512]; row 64 is
      the denominator. Normalization: reciprocal -> K=1 broadcast matmul
      -> elementwise multiply, written to zT bf16.
  out = sum_pairs zT_pair.T @ WO_pair -> [2048, 768] fp32, DMA'd out.

Biases: b_K provably cancels in softmax (it shifts every score in a row
by the same amount). b_V and b_O contribute sum_h b_V[h] @ W_O[h] + b_O,
a constant row added host-side. A nonzero b_Q would need a device-side
per-key score offset; inputs here always have b_Q = 0, so that case (and
any unexpected shape) falls back to a numpy reference implementation.
"""
import os
import sys
from collections import deque

sys.path.insert(0, "/opt/trn_rl_repo")

import numpy as np
import ml_dtypes

D_MODEL, N_HEADS, D_HEAD = 768, 12, 64
BATCH, SEQ = 4, 2048
HPG = 6           # heads per group (per core)
NPAIR = HPG // 2  # head pairs per core
NCORES = 8
QC = 512          # q chunk (moving operand width)
KT_TILES = SEQ // 128
QC_TILES = SEQ // QC
MT = D_MODEL // 128  # contraction tiles for projections
BF16 = ml_dtypes.bfloat16

_prog_cache = {}


def _numpy_ref(normalized_resid_pre, W_Q, W_K, W_V, W_O, b_Q, b_K, b_V, b_O):
    x = normalized_resid_pre.astype(np.float32)
    Q = np.einsum("bsm,hmd->bshd", x, W_Q) + b_Q
    K = np.einsum("bsm,hmd->bshd", x, W_K) + b_K
    V = np.einsum("bsm,hmd->bshd", x, W_V) + b_V
    scores = np.einsum("bqhd,bkhd->bhqk", Q, K) / np.sqrt(np.float32(W_Q.shape[-1]))
    s = x.shape[1]
    causal = np.tril(np.ones((s, s), dtype=bool))
    scores = np.where(causal, scores, -np.inf)
    scores -= scores.max(axis=-1, keepdims=True)
    e = np.exp(scores)
    probs = e / e.sum(axis=-1, keepdims=True)
    z = np.einsum("bkhd,bhqk->bqhd", V, probs)
    return (np.einsum("bqhd,hdm->bqm", z, W_O) + b_O).astype(np.float32)


def _build_program():
    from concourse import bacc, tile
    import concourse.bass as bass
    import concourse.mybir as mybir

    f32 = mybir.dt.float32
    bf16 = mybir.dt.bfloat16

    nc = bacc.Bacc(None)
    xT_d = nc.dram_tensor("xT", [D_MODEL, SEQ], bf16, kind="ExternalInput")
    wqkv_d = nc.dram_tensor("wqkv", [D_MODEL, 3 * HPG * D_HEAD], bf16, kind="ExternalInput")
    wo_d = nc.dram_tensor("wo", [HPG * D_HEAD, D_MODEL], bf16, kind="ExternalInput")
    mask_d = nc.dram_tensor("mask", [128, 128], bf16, kind="ExternalInput")
    out_d = nc.dram_tensor("out", [SEQ, D_MODEL], f32, kind="ExternalOutput")
    recip_d = nc.dram_tensor("recip_scratch", [HPG * (SEQ // QC), QC], f32)

    with tile.TileContext(nc) as tc:
        with (
            tc.tile_pool(name="persist", bufs=1) as persist,
            tc.tile_pool(name="expsb", bufs=8) as expsb,
            tc.tile_pool(name="rbsb", bufs=4) as rbsb,
            tc.tile_pool(name="outsb", bufs=3) as outsb,
            tc.tile_pool(name="dtmpsb", bufs=8) as dtmpsb,
            tc.tile_pool(name="ps_big", bufs=3, space="PSUM") as ps_big,
            tc.tile_pool(name="ps_z", bufs=2, space="PSUM") as ps_z,
        )    :
            # ---- persistent SBUF tiles ----
            xT = [persist.tile([128, SEQ], bf16, tag=f"xT{i}", name=f"xT{i}") for i in range(MT)]
            wqkv = [persist.tile([128, 3 * HPG * D_HEAD], bf16, tag=f"wqkv{i}", name=f"wqkv{i}") for i in range(MT)]
            wq = [wqkv[i][:, 0 : HPG * D_HEAD] for i in range(MT)]
            wk = [wqkv[i][:, HPG * D_HEAD : 2 * HPG * D_HEAD] for i in range(MT)]
            wv = [wqkv[i][:, 2 * HPG * D_HEAD : 3 * HPG * D_HEAD] for i in range(MT)]
            wo = [persist.tile([128, D_MODEL], bf16, tag=f"wo{i}", name=f"wo{i}") for i in range(NPAIR)]
            QTz = [persist.tile([128, SEQ], bf16, tag=f"QTz{i}", name=f"QTz{i}") for i in range(HPG)]
            KT = [persist.tile([128, SEQ], bf16, tag=f"KT{i}", name=f"KT{i}") for i in range(NPAIR)]
            zT = [persist.tile([128, SEQ], bf16, tag=f"zT{i}", name=f"zT{i}") for i in range(NPAIR)]
            V = [persist.tile([128, HPG, D_HEAD + 1], bf16, tag=f"V{i}", name=f"V{i}") for i in range(KT_TILES)]
            mask01 = persist.tile([128, 128], bf16, tag="mask01")
            ones64 = persist.tile([1, 64], bf16, tag="ones64")

            # ---- input DMAs. dma_start costs ~0.6us of ISSUE time on the
            # issuing sequencer, so the loads are round-robined across the
            # sync/vector/scalar sequencers (3x issue rate); xT+wq first so
            # the first projection groups start ~10us in. QTz zero-halves
            # are memset on the otherwise-idle gpsimd engine. ----
            dmas = []
            for i in range(MT):
                dmas.append((wqkv[i], wqkv_d[128 * i : 128 * (i + 1), :]))
                dmas.append((xT[i], xT_d[128 * i : 128 * (i + 1), :]))
            dmas.append((mask01, mask_d[:, :]))
            for p in range(NPAIR):
                dmas.append((wo[p], wo_d[128 * p : 128 * (p + 1), :]))
            issuers = [nc.sync, nc.scalar, nc.gpsimd]
            for idx, (dst, srcap) in enumerate(dmas):
                if idx < 12:
                    issuers[idx % 3].dma_start(out=dst, in_=srcap)
                else:
                    issuers[idx % 2].dma_start(out=dst, in_=srcap)
            for h in range(HPG):
                r0 = 64 * (h % 2)
                nc.gpsimd.memset(QTz[h][64 - r0 : 128 - r0, :], 0.0)
            for kt in range(KT_TILES):
                nc.gpsimd.memset(V[kt][:, :, D_HEAD : D_HEAD + 1], 1.0)
            nc.gpsimd.memset(ones64, 1.0)
            # dummy exp so the ACT table load (~2.7us) overlaps input DMAs
            # instead of stalling the first attention exp
            warm = persist.tile([1, 1], f32, tag="warm")
            nc.vector.memset(warm, 0.0)
            nc.scalar.activation(out=warm, in_=warm,
                                 func=mybir.ActivationFunctionType.Exp, scale=1.0)

            # ---- emission helpers ----
            def emit_qk_chunk(p, j):
                    cols = slice(128 * p, 128 * (p + 1))
                    qs = slice(QC * j, QC * (j + 1))
                    psq = ps_big.tile([128, QC], f32, tag="big", name="psq")
                    for m in range(MT):
                        nc.tensor.matmul(psq, lhsT=wq[m][:, cols], rhs=xT[m][:, qs],
                                         start=(m == 0), stop=(m == MT - 1))
                    nc.vector.tensor_copy(QTz[2 * p][0:64, qs], psq[0:64, :])
                    nc.vector.tensor_copy(QTz[2 * p + 1][64:128, qs], psq[64:128, :])
                    psk = ps_big.tile([128, QC], f32, tag="big", name="psk")
                    for m in range(MT):
                        nc.tensor.matmul(psk, lhsT=wk[m][:, cols], rhs=xT[m][:, qs],
                                         start=(m == 0), stop=(m == MT - 1))
                    nc.vector.tensor_copy(KT[p][:, qs], psk)

            def emit_v(kts):
                for kt in kts:
                    ks = slice(128 * kt, 128 * (kt + 1))
                    psv = ps_big.tile([128, HPG * D_HEAD], f32, tag="big", name="psv")
                    for m in range(MT):
                        nc.tensor.matmul(psv, lhsT=xT[m][:, ks], rhs=wv[m],
                                         start=(m == 0), stop=(m == MT - 1))
                    nc.vector.tensor_copy(
                        V[kt][:, :, 0:D_HEAD],
                        psv.rearrange("p (h d) -> p h d", h=HPG))

            def emit_scores(h, j, kt2):
                p = h // 2
                pss = ps_big.tile([128, 2 * QC], f32, tag="big", name="pss")
                off0 = 0
                for u in (0, 1):
                    kt = kt2 + u
                    delta = kt - 4 * j  # >=0 on diagonal blocks
                    off = 128 * delta if delta >= 0 else 0
                    if u == 0:
                        off0 = off
                    nc.tensor.matmul(
                        pss[:, QC * u + off : QC * (u + 1)],
                        lhsT=KT[p][:, 128 * kt : 128 * (kt + 1)],
                        rhs=QTz[h][:, QC * j + off : QC * (j + 1)],
                        start=True, stop=True,
                        skip_group_check=True,
                    )
                expt = expsb.tile([128, 2 * QC], bf16, tag="exp", name="expt")
                nc.scalar.activation(out=expt[:, off0:], in_=pss[:, off0:],
                                     func=mybir.ActivationFunctionType.Exp,
                                     scale=0.125)
                for u in (0, 1):
                    delta = kt2 + u - 4 * j
                    if delta >= 0:
                        off = 128 * delta
                        blk = slice(QC * u + off, QC * u + off + 128)
                        nc.vector.tensor_mul(expt[:, blk], expt[:, blk], mask01)
                return expt

            def emit_pv(h, j, psz, nkt, kt2, expt):
                for u in (0, 1):
                    kt = kt2 + u
                    delta = kt - 4 * j
                    off = 128 * delta if delta >= 0 else 0
                    nc.tensor.matmul(
                        psz[:, off:QC],
                        lhsT=V[kt][:, h, :],
                        rhs=expt[:, QC * u + off : QC * (u + 1)],
                        start=(kt == 0), stop=(kt == nkt - 1),
                        skip_group_check=True,
                    )

            def emit_norm(h, j, psz):
                # per-head normalization chain (approx reciprocal -> DRAM
                # hop -> partition-broadcast DMA -> multiply); hides behind
                # subsequent attention work
                p, r0 = h // 2, 64 * (h % 2)
                qs = slice(QC * j, QC * (j + 1))
                row = HPG * j + h
                dtmp = dtmpsb.tile([1, QC], f32, tag="dtmp", name="dtmp")
                nc.vector.tensor_copy(dtmp, psz[D_HEAD : D_HEAD + 1, :])
                rtmp = dtmpsb.tile([1, QC], f32, tag="rtmp", name="rtmp")
                nc.vector.reciprocal_approx_fast(rtmp, dtmp)
                nc.gpsimd.dma_start(out=recip_d[row : row + 1, :], in_=rtmp)
                nc.vector.tensor_copy(zT[p][r0 : r0 + 64, qs], psz[0:D_HEAD, :])
                sl = recip_d[row : row + 1, :]
                rb = rbsb.tile([128, QC], f32, tag="rb", name="rb")
                nc.gpsimd.dma_start(
                    out=rb[r0 : r0 + 64, :],
                    in_=bass.AP(tensor=sl.tensor, offset=sl.offset,
                                ap=[[0, D_HEAD]] + list(sl.ap[-1:])))
                nc.vector.tensor_mul(zT[p][r0 : r0 + 64, qs],
                                     zT[p][r0 : r0 + 64, qs],
                                     rb[r0 : r0 + 64, :])

            def emit_norm_fast(h, j, psz):
                # tail-only normalization: reciprocal -> bf16 -> K=1 ones
                # matmul broadcasts it across 64 partitions in PSUM -> mul.
                # ~2.5us chain vs ~5-6us for the DMA-roundtrip variant; used
                # for the last pair so outproj(3) starts sooner.
                p, r0 = h // 2, 64 * (h % 2)
                qs = slice(QC * j, QC * (j + 1))
                dtmp = dtmpsb.tile([1, QC], f32, tag="dtmp", name="dtmp")
                nc.vector.tensor_copy(dtmp, psz[D_HEAD : D_HEAD + 1, :])
                rtf = dtmpsb.tile([1, QC], f32, tag="rtmp", name="rtf")
                nc.vector.reciprocal_approx_fast(rtf, dtmp)
                rtb = dtmpsb.tile([1, QC], bf16, tag="rtb", name="rtb")
                nc.vector.tensor_copy(rtb, rtf)
                psbr = ps_big.tile([128, QC], f32, tag="big", name="psbr")
                nc.tensor.matmul(psbr[r0 : r0 + 64, :], lhsT=ones64, rhs=rtb,
                                 start=True, stop=True, skip_group_check=True)
                nc.vector.tensor_copy(zT[p][r0 : r0 + 64, qs], psz[0:D_HEAD, :])
                nc.vector.tensor_mul(zT[p][r0 : r0 + 64, qs],
                                     zT[p][r0 : r0 + 64, qs],
                                     psbr[r0 : r0 + 64, :])

            def emit_attention(h, j, carry):
                # k-loop with scores staggered two k-pairs ahead of PV. The
                # tail PVs + normalization are returned via `carry` as
                # closures and emitted inside the NEXT unit's score stream,
                # so the PE never drains waiting on the freshest exps at a
                # unit boundary (cross-unit software pipelining).
                nkt = 4 * j + 4
                psz = ps_z.tile([D_HEAD + 1, QC], f32, tag="z", name="psz")
                pend = deque()
                for kt2 in range(0, nkt, 2):
                    expt = emit_scores(h, j, kt2)
                    pend.append((kt2, expt))
                    if carry:
                        carry.popleft()()
                    elif len(pend) > 2:
                        kt2p, exptp = pend.popleft()
                        emit_pv(h, j, psz, nkt, kt2p, exptp)

                def mk_pv(kt2p, exptp):
                    return lambda: emit_pv(h, j, psz, nkt, kt2p, exptp)

                while pend:
                    carry.append(mk_pv(*pend.popleft()))
                if j == QC_TILES - 1 and h >= HPG - 2:
                    carry.append(lambda: emit_norm_fast(h, j, psz))
                else:
                    carry.append(lambda: emit_norm(h, j, psz))

            def emit_outproj_ctile(c):
                    cs = slice(128 * c, 128 * (c + 1))
                    pso = ps_big.tile([128, D_MODEL], f32, tag="big", name="pso")
                    for p in range(NPAIR):
                        nc.tensor.matmul(pso[:, 0:512], lhsT=zT[p][:, cs],
                                         rhs=wo[p][:, 0:512],
                                         start=(p == 0), stop=(p == NPAIR - 1))
                        nc.tensor.matmul(pso[:, 512:768], lhsT=zT[p][:, cs],
                                         rhs=wo[p][:, 512:768],
                                         start=(p == 0), stop=(p == NPAIR - 1))
                    outt = outsb.tile([128, D_MODEL], f32, tag="out", name="outt")
                    nc.vector.tensor_copy(outt, pso)
                    nc.sync.dma_start(out=out_d[cs, :], in_=outt)

            def emit_outproj(j):
                for c in range(4 * j, 4 * (j + 1)):
                    emit_outproj_ctile(c)

            # ---- schedule ----
            # The attention chunks are increasingly exp(ACT)-bound (chunk j
            # has (j+1)*11.8us of exp vs (j+1)*~7us of attention matmul), so
            # projection and output-projection work is DEFERRED into the
            # later chunks as head-slot ballast: chunk-2 Q/K projections run
            # during chunk 1, chunk-3 Q/K during chunk 2, V chunks 2/3 at the
            # start of their own chunk, and all 12 outproj tiles of chunks
            # 0-2 spread across chunk 3. Unit internals match the v1 texture
            # (coarse bursts, not fine pacing -- per-instruction sync costs
            # ~40ns/boundary on this machine).
            carry = deque()

            def drain_carry():
                while carry:
                    carry.popleft()()

            # j0 phase: chunks 0/1 projections + V[0..7] + j=0 attention.
            # V tiles a unit's PVs read must be EMITTED before the unit
            # (engine queues execute in program order).
            emit_qk_chunk(0, 0)
            emit_v([0, 1])
            emit_qk_chunk(1, 0)
            emit_v([2, 3])
            emit_attention(0, 0, carry)
            emit_attention(1, 0, carry)
            emit_qk_chunk(2, 0)
            drain_carry()
            emit_attention(2, 0, carry)
            emit_attention(3, 0, carry)
            emit_qk_chunk(0, 1)
            emit_v([4, 5])
            drain_carry()
            emit_attention(4, 0, carry)
            emit_attention(5, 0, carry)
            emit_qk_chunk(1, 1)
            emit_qk_chunk(2, 1)
            emit_v([6, 7])
            drain_carry()

            # ballast closures per (j, h) slot
            def qk(p, j):
                return lambda: emit_qk_chunk(p, j)

            def vg(kts):
                return lambda: emit_v(kts)

            def op(c):
                return lambda: emit_outproj_ctile(c)

            SLOTS = {
                (1, 0): [qk(0, 2)], (1, 1): [qk(1, 2)], (1, 2): [qk(2, 2)],
                (2, 1): [qk(0, 3)], (2, 2): [qk(1, 3)], (2, 3): [qk(2, 3)],
                (2, 4): [op(0)], (2, 5): [op(1)],
                (3, 1): [op(2), op(3)], (3, 2): [op(4), op(5)],
                (3, 3): [op(6), op(7)], (3, 4): [op(8), op(9)],
            }
            # deferred V projections ride the carry queue: they pop inside
            # the first steps of the chunk's first unit, well before any PV
            # that reads them is emitted
            PRE = {2: [vg([8, 9]), vg([10, 11])],
                   3: [vg([12, 13]), vg([14, 15])]}
            for j in range(1, QC_TILES):
                carry.extend(PRE.get(j, []))
                for h in range(HPG):
                    emit_attention(h, j, carry)
                    for fn in SLOTS.get((j, h), []):
                        fn()
            # tail: the last two outproj(2) tiles bridge the final norm
            # chains so the PE stays warm into outproj(3)
            for c in (10, 11):
                emit_outproj_ctile(c)
                for _ in range(2):
                    if carry:
                        carry.popleft()()
            drain_carry()
            emit_outproj(QC_TILES - 1)

    nc.finalize()
    return nc


def kernel(**inputs):
    x = inputs["normalized_resid_pre"]
    W_Q, W_K, W_V, W_O = inputs["W_Q"], inputs["W_K"], inputs["W_V"], inputs["W_O"]
    b_Q, b_K, b_V, b_O = inputs["b_Q"], inputs["b_K"], inputs["b_V"], inputs["b_O"]

    expected = (
        x.shape == (BATCH, SEQ, D_MODEL)
        and W_Q.shape == (N_HEADS, D_MODEL, D_HEAD)
        and W_K.shape == (N_HEADS, D_MODEL, D_HEAD)
        and W_V.shape == (N_HEADS, D_MODEL, D_HEAD)
        and W_O.shape == (N_HEADS, D_HEAD, D_MODEL)
        and not np.any(b_Q)
    )
    if not expected:
        return _numpy_ref(**inputs)

    from concourse.bass_utils import run_bass_kernel_spmd

    if "nc" not in _prog_cache:
        _prog_cache["nc"] = _build_program()
    nc = _prog_cache["nc"]

    # host-side prep: transpose + cast + pack per head-group
    xT = np.ascontiguousarray(x.transpose(0, 2, 1)).astype(BF16)  # [B, 768, 2048]
    # b_K shifts every score in a softmax row equally -> cancels exactly.
    groups = []
    for g in range(2):
        hs = slice(HPG * g, HPG * (g + 1))
        wqkv = np.concatenate(
            [W.transpose(1, 0, 2).reshape(D_MODEL, HPG * D_HEAD)
             for W in (W_Q[hs], W_K[hs], W_V[hs])], axis=1)
        groups.append({
            "wqkv": np.ascontiguousarray(wqkv).astype(BF16),
            "wo": np.ascontiguousarray(W_O[hs].reshape(HPG * D_HEAD, D_MODEL)).astype(BF16),
        })
    ii, jj = np.arange(128)[:, None], np.arange(128)[None, :]
    mask = np.where(jj >= ii, np.float32(1.0), np.float32(0.0)).astype(BF16)

    in_maps = []
    for c in range(NCORES):
        b, g = c // 2, c % 2
        m = {"xT": xT[b], "mask": mask}
        m.update(groups[g])
        in_maps.append(m)

    trace = bool(os.environ.get("ATTN_KERNEL_TRACE"))
    res = run_bass_kernel_spmd(nc, in_maps, list(range(NCORES)), trace=trace)
    _prog_cache["last_exec_time_ns"] = res.exec_time_ns
    _prog_cache["last_results"] = res

    # b_V/b_O fold into a constant row (softmax weights sum to 1).
    const_row = np.einsum("hd,hdm->m", b_V.astype(np.float64), W_O.astype(np.float64))
    const_row = (const_row + b_O.astype(np.float64)).astype(np.float32)

    out = np.empty((BATCH, SEQ, D_MODEL), dtype=np.float32)
    for b in range(BATCH):
        out[b] = res.results[2 * b]["out"] + res.results[2 * b + 1]["out"] + const_row
    return out

